# revision 29
# baseline (speedup 1.0000x reference)
"""Bass/Trainium2 kernel for nn_DotsGenerator (scatter_memory).

Strategy (8 NeuronCores, SPMD), v7 "bf16, aligned M-dup":
  - 512 crops sharded 64/core along the crop axis (host slices zero-haloed
    42x42 bf16 crops out of the image).
  - Crop loads: batched contiguous-strip DMAs build the conv1 im2col
    directly (shift baked into the source offset), 4 crops per group.
  - conv1 (bf16, K=27): 4 matmuls per crop, output M-DUPLICATED to rows
    51-101 (aligned dup); evacuations write a 2-bank bf16 pad ring: bank1
    (rows 0-50) standard halo layout, bank2 (rows 51-101) shifted one
    column left.  Borders are pre-initialized ONCE via a DMA'd constant.
  - conv2 (bf16, pixel-moving): 6 paired-shift passes of K=102 per 10-row
    chunk; the stationary w2 is M-duplicated so even pixels evacuate from
    psum rows 0-50 and odd pixels from rows 51-101 STRAIGHT into the ft
    layout -- partition-aligned, conv2 bias riding on the per-partition
    activation bias.
  - ft: [102 = (parity, ch), 800 px-pairs x 16 crops] bf16, 2-group ring.
  - conv3 (bf16, K=102): 800 accumulating matmuls of N=16 per 16-crop
    group, run as soon as the group's conv2 lands (no tail); w3 is
    restaged host-side so each load is a contiguous [102, 2550] DMA and
    stays resident in SBUF.
  - Each core also DMA-copies its 135-row band of the image to its output
    (8 chunks spread over the crop loop).
  - Host assembles the bands and applies the 512*17*9 dot values.
"""

import sys

sys.path.insert(0, "/opt/trn_rl_repo")

import numpy as np
import ml_dtypes

import concourse.bass as bass
import concourse.bacc as bacc
import concourse.tile as tile
import concourse.mybir as mybir
from concourse.bass_utils import run_bass_kernel_spmd

F32 = mybir.dt.float32
BF16 = mybir.dt.bfloat16

NCORES = 8
NGT = 512
PC = NGT // NCORES  # crops per core = 64
CROP = 40
PAD = 42            # padded map 42x42
PIX = CROP * CROP   # 1600
NJ = PIX // 2       # 800 pixel pairs
BAND_H = 1080 // NCORES  # 135 rows of output per core
IMG_H, IMG_W = 1080, 1920
EPS = 1e-5
NCH = 51
KB = 64             # second bank base partition (32-aligned)
K2 = KB + NCH       # 115: two banks / parity-channel rows
HALO = PAD * PAD    # 1764 elems per haloed crop channel
STRIP = (CROP - 1) * PAD + CROP  # 1678 contiguous elems cover a shifted window
GRP = 4             # crops loaded per batched DMA group
CG = 16             # conv3 crop-group size (ft ring granularity)
NCGRP = PC // CG    # 4 conv3 groups
W3LD = 32           # w3 DMA block loads (small: don't hog DMA engines)

DOT_LIST = np.array(
    [(30, 20), (20, 30), (10, 20), (20, 10), (40, 20), (34, 34), (20, 40),
     (6, 34), (0, 20), (6, 6), (20, 0), (34, 6), (17, 20), (23, 20),
     (20, 17), (20, 23), (20, 20)], dtype=np.int64)  # [17,2] (dy,dx)
DIRS = np.array([(dy, dx) for dy in (-1, 0, 1) for dx in (-1, 0, 1)],
                dtype=np.int64)  # [9,2]

AF = mybir.ActivationFunctionType
ALU = mybir.AluOpType


def _emit(ctx, tc, io):
    nc = tc.nc
    crops = io["crops"]        # [3, PC, 42, 42] bf16 (zero halo)
    w1r = io["w1r"]            # [27, 102]  bf16 (im2col lhsT, M-dup at 51)
    w2r = io["w2r"]            # [102, 6*102] bf16 (2-bank K, M-dup out)
    w3r = io["w3r"]            # [102, 800*51] bf16 ((parity, ch), j, out)
    b1 = io["b1"]              # [128, 1] f32 (rows 0-50 and 51-101)
    b2 = io["b2"]              # [128, 1] f32
    b3 = io["b3"]              # [128, 1] f32
    padinit = io["padinit"]    # [128, 1764] bf16 zeros
    ftinit = io["ftinit"]      # [13, NJ*CG] bf16 zeros
    vals_out = io["vals_out"]  # [51, PC] f32 out
    band_src = io["band_src"]  # [3, BAND_H, 1920] f32
    out_band = io["out_band"]  # [3, BAND_H, 1920] f32 out

    consts = ctx.enter_context(tc.tile_pool(name="consts", bufs=1))
    cin_pool = ctx.enter_context(tc.tile_pool(name="cin", bufs=3))
    ps1_pool = ctx.enter_context(tc.tile_pool(name="ps1", bufs=2,
                                              space="PSUM"))  # 2 banks each
    ps2_pool = ctx.enter_context(tc.tile_pool(name="ps2", bufs=3,
                                              space="PSUM"))  # 1 bank each
    ps3_pool = ctx.enter_context(tc.tile_pool(name="ps3", bufs=1,
                                              space="PSUM"))

    # ---- persistent SBUF state ----
    w1t = consts.tile([128, K2], BF16)
    nc.sync.dma_start(w1t[0:27, :], w1r[:, :])
    b1t = consts.tile([128, 1], F32)
    nc.sync.dma_start(b1t[:, :], b1[:, :])
    w2t = consts.tile([128, 6 * K2], BF16)
    b2t = consts.tile([128, 1], F32)
    b3t = consts.tile([128, 1], F32)
    w3t = consts.tile([K2, NJ * NCH], BF16)
    w3t_v = w3t.rearrange("p (j o) -> p j o", j=NJ)
    vals = consts.tile([128, PC], F32)
    pads = [consts.tile([128, HALO], BF16, name=f"pad{k}") for k in range(4)]
    fts = [consts.tile([K2, NJ * CG], BF16, name=f"ft{k}") for k in range(2)]

    def emit_init():
        # everything conv1(0) does NOT need -- emitted after the first crop
        # group's strip loads so the PE can start ~14us earlier.
        nc.scalar.dma_start(w2t[0:K2, :], w2r[:, :])
        nc.scalar.dma_start(b2t[:, :], b2[:, :])
        nc.scalar.dma_start(b3t[:, :], b3[:, :])
        for k in range(4):
            # one-shot halo init; per-crop evacs only touch the interior
            nc.scalar.dma_start(pads[k][:, :], padinit[:, :])
        for k in range(2):
            # zero the dead K-rows once: 0-weight x NaN poisons psum
            nc.scalar.dma_start(fts[k][NCH:KB, :], ftinit[:, :])

    shifts = [(ky, kx) for ky in range(3) for kx in range(3)]
    crops_flat = crops.rearrange("c n h w -> c n (h w)")
    w3_jblk = NJ // W3LD  # j's per w3 load

    def emit_load(g):
        cin = cin_pool.tile([128, GRP * HALO], BF16, tag="cin")
        cin_v = cin.rearrange("p (n j) -> p n j", n=GRP)
        csrc = crops_flat[:, g * GRP:(g + 1) * GRP, :]
        for s, (ky, kx) in enumerate(shifts):
            off = ky * PAD + kx
            nc.sync.dma_start(cin_v[3 * s:3 * s + 3, :, 0:STRIP],
                              csrc[:, :, off:off + STRIP])
        return cin.rearrange("p (n h w) -> p n h w", n=GRP, h=PAD)

    def emit_conv1(c, cin4):
        # bf16 K=27, M-duplicated out (rows 0-50 and 51-101); the duplicate
        # is evacuated one column left so conv2 reads tap kx on rows 0-50
        # and tap kx+1 on rows 51-101 with one AP.  Two matmuls (10 rows
        # each) land in the two banks of one psum tile so each evacuation
        # covers 20 rows in a single instruction.
        pad_v = pads[c % 4].rearrange("p (h w) -> p h w", h=PAD)
        for q in range(2):
            r0 = 20 * q
            ps1 = ps1_pool.tile([128, 1024], F32, tag="ps1")
            ps1_g = ps1.rearrange("p (g x) -> p g x", g=2)
            for h in range(2):
                nc.tensor.matmul(
                    ps1[0:K2, 512 * h:512 * h + 400], w1t[0:27, 0:K2],
                    cin4[0:27, c % GRP, r0 + 10 * h:r0 + 10 * h + 10, 0:CROP],
                    start=True, stop=True)
            nc.scalar.activation(pad_v[0:NCH, 1 + r0:21 + r0, 1:41],
                                 ps1_g[0:NCH, :, 0:400], AF.Relu,
                                 bias=b1t[0:NCH, 0:1])
            nc.vector.tensor_scalar(pad_v[KB:K2, 1 + r0:21 + r0, 0:40],
                                    ps1_g[KB:K2, :, 0:400],
                                    b1t[KB:K2, 0:1], 0.0,
                                    ALU.add, ALU.max)

    def emit_conv2(cc, hs):
        # bf16 pixel-moving: 6 paired-shift passes of K=102 per 10-row
        # chunk; M-duplicated out so even/odd pixels evacuate partition-
        # aligned into the ft layout.
        pad_v = pads[cc % 4].rearrange("p (h w) -> p h w", h=PAD)
        g, i = divmod(cc, CG)
        ftv = fts[g % 2].rearrange("p (j n) -> p j n", j=NJ)
        for h in hs:  # chunks of 10 out-rows, one psum bank each
            r0 = 10 * h
            ps2 = ps2_pool.tile([128, 512], F32, tag="ps2")
            ps2_v = ps2.rearrange("p (j r) -> p j r", r=2)
            for s, (ky, kxg) in enumerate(
                    (ky, kx) for ky in range(3) for kx in (0, 2)):
                nc.tensor.matmul(
                    ps2[0:K2, 0:400],
                    w2t[0:K2, s * K2:(s + 1) * K2],
                    pad_v[0:K2, r0 + ky:r0 + ky + 10, kxg:kxg + 40],
                    start=(s == 0), stop=(s == 5))
            nc.scalar.activation(ftv[0:NCH, 200 * h:200 * h + 200, i],
                                 ps2_v[0:NCH, 0:200, 0], AF.Relu,
                                 bias=b2t[0:NCH, 0:1])
            nc.vector.tensor_scalar(ftv[KB:K2, 200 * h:200 * h + 200, i],
                                    ps2_v[KB:K2, 0:200, 1],
                                    b2t[KB:K2, 0:1], 0.0,
                                    ALU.add, ALU.max)

    C3CH = 50  # conv3 j's emitted per crop slot (spread, no burst)
    cur_ps3 = [None]

    def emit_conv3_chunk(g, j0):
        ftv = fts[g % 2].rearrange("p (j n) -> p j n", j=NJ)
        if j0 == 0:
            cur_ps3[0] = ps3_pool.tile([128, CG], F32, tag="ps3",
                                       name="ps3")
        ps3 = cur_ps3[0]
        for j in range(j0, j0 + C3CH):
            nc.tensor.matmul(ps3[0:NCH, :], w3t_v[0:K2, j, :],
                             ftv[0:K2, j, :],
                             start=(j == 0), stop=(j == NJ - 1),
                             skip_group_check=True)
        if j0 + C3CH >= NJ:
            nc.scalar.activation(vals[0:NCH, g * CG:(g + 1) * CG],
                                 ps3[0:NCH, :], AF.Relu,
                                 bias=b3t[0:NCH, 0:1])

    skip12 = ("no_conv1" in DBG) or ("no_conv2" in DBG)
    skip3 = "no_conv3" in DBG
    cin4 = None
    c3q = []  # pending conv3 (group, j0) chunks, drained one per crop slot
    # software pipeline: the first half of conv2(c-1) is emitted, then
    # conv1(c), then the rest -- the conv1 matmuls cover the evacuation
    # latency of the early conv2 chunks so the PE never drains dry.
    for c in range(PC + 2):
        if c < PC:
            if c % GRP == 0:
                cin4 = emit_load(c // GRP)
            if c == 0:
                emit_init()
            if c < W3LD:
                nc.sync.dma_start(
                    w3t[:, c * w3_jblk * NCH:(c + 1) * w3_jblk * NCH],
                    w3r[:, c * w3_jblk * NCH:(c + 1) * w3_jblk * NCH])
            if c >= 32 and c % 4 == 1 and "no_band" not in DBG:
                k = (c - 32) // 4
                a, bnd = 17 * k, min(17 * k + 17, BAND_H)
                nc.sync.dma_start(out_band[:, a:bnd, :],
                                  band_src[:, a:bnd, :])
            if c >= 1 and not skip12:
                emit_conv2(c - 1, (0, 1))
            if not skip12:
                emit_conv1(c, cin4)
            if c >= 1 and not skip12:
                emit_conv2(c - 1, (2, 3))
        elif c == PC:
            if not skip12:
                emit_conv2(c - 1, (0, 1, 2, 3))
        if c >= 2 and not skip12 and not skip3:
            if (c - 2) % CG == CG - 1:
                g3 = (c - 2) // CG
                c3q.extend((g3, j0) for j0 in range(0, NJ, C3CH))
        if c3q and not skip12 and not skip3:
            emit_conv3_chunk(*c3q.pop(0))

    while c3q and not skip12 and not skip3:
        emit_conv3_chunk(*c3q.pop(0))
    if skip3 or skip12:
        nc.gpsimd.memset(vals[:, :], 0.0)
    nc.vector.tensor_scalar_min(vals[0:NCH, :], vals[0:NCH, :], 255.0)
    nc.sync.dma_start(vals_out[:, :], vals[0:NCH, :])


_CACHE = {}
DBG = set()          # ablation flags for cost-model analysis
RUN_KWARGS = {}      # test harness may set {"trace": True} for profiling
LAST_RESULTS = None


def _build():
    if "nc" in _CACHE:
        return _CACHE["nc"]
    nc = bacc.Bacc("TRN2", target_bir_lowering=False, debug=False,
                   num_devices=NCORES)
    io = {
        "crops": nc.dram_tensor("crops", [3, PC, PAD, PAD], BF16,
                                kind="ExternalInput").ap(),
        "w1r": nc.dram_tensor("w1r", [27, K2], BF16,
                              kind="ExternalInput").ap(),
        "w2r": nc.dram_tensor("w2r", [K2, 6 * K2], BF16,
                              kind="ExternalInput").ap(),
        "w3r": nc.dram_tensor("w3r", [K2, NJ * NCH], BF16,
                              kind="ExternalInput").ap(),
        "b1": nc.dram_tensor("b1", [128, 1], F32, kind="ExternalInput").ap(),
        "b2": nc.dram_tensor("b2", [128, 1], F32, kind="ExternalInput").ap(),
        "b3": nc.dram_tensor("b3", [128, 1], F32, kind="ExternalInput").ap(),
        "padinit": nc.dram_tensor("padinit", [128, HALO], BF16,
                                  kind="ExternalInput").ap(),
        "ftinit": nc.dram_tensor("ftinit", [KB - NCH, NJ * CG], BF16,
                                 kind="ExternalInput").ap(),
        "band_src": nc.dram_tensor("band_src", [3, BAND_H, IMG_W], F32,
                                   kind="ExternalInput").ap(),
        "vals_out": nc.dram_tensor("vals_out", [NCH, PC], F32,
                                   kind="ExternalOutput").ap(),
        "out_band": nc.dram_tensor("out_band", [3, BAND_H, IMG_W], F32,
                                   kind="ExternalOutput").ap(),
    }
    from contextlib import ExitStack
    with tile.TileContext(nc) as tc, ExitStack() as ctx:
        _emit(ctx, tc, io)
    nc.compile()
    _CACHE["nc"] = nc
    return nc


def _fold(w, g, b, m, v):
    scale = g / np.sqrt(v + EPS)
    return w * scale[:, None, None, None], (b - m * scale).astype(np.float32)


def _bf16(a):
    return np.ascontiguousarray(a).astype(ml_dtypes.bfloat16)


def _prep_weights(w1, g1, b1, m1, v1, w2, g2, b2, m2, v2, w3, g3, b3, m3, v3):
    w1f, b1f = _fold(w1, g1, b1, m1, v1)  # [51,3,3,3]
    w2f, b2f = _fold(w2, g2, b2, m2, v2)  # [51,51,3,3]
    w3f, b3f = _fold(w3, g3, b3, m3, v3)  # [51,51,40,40]
    # conv1 im2col lhsT rows (s=ky*3+kx, ch) -> [27, 102] M-dup at 51
    w1c = w1f.transpose(2, 3, 1, 0).reshape(27, NCH)
    w1r = np.zeros((27, K2), np.float32)
    w1r[:, 0:NCH] = w1c
    w1r[:, KB:K2] = w1c
    # conv2 stationary [102 K, 6 passes x 102 M]: K rows 0-50 read bank1
    # (tap kxg), rows 51-101 read bank2 (tap kxg+1); M cols 0-50 and
    # 51-101 both carry the out-channels (even/odd evac duplicate).
    w2c = w2f.transpose(2, 3, 1, 0)  # [ky, kx, ci, o]
    w2r = np.zeros((K2, 6, K2), np.float32)
    for ky in range(3):
        for kxg_i, kxg in enumerate((0, 2)):
            s = ky * 2 + kxg_i
            for mo in (0, KB):
                w2r[0:NCH, s, mo:mo + NCH] = w2c[ky, kxg]
                if kxg + 1 < 3:
                    w2r[KB:K2, s, mo:mo + NCH] = w2c[ky, kxg + 1]
    # conv3 [102 = (parity, ch) K-rows, j, o]
    w3p = w3f.transpose(1, 2, 3, 0).reshape(NCH, NJ, 2, NCH)  # [ci,j,par,o]
    w3r = np.zeros((K2, NJ, NCH), np.float32)
    w3r[0:NCH] = w3p[:, :, 0, :]
    w3r[KB:K2] = w3p[:, :, 1, :]
    b1v = np.zeros((128, 1), np.float32)
    b1v[0:NCH, 0] = b1f
    b1v[KB:K2, 0] = b1f
    b2v = np.zeros((128, 1), np.float32)
    b2v[0:NCH, 0] = b2f
    b2v[KB:K2, 0] = b2f
    b3v = np.zeros((128, 1), np.float32)
    b3v[0:NCH, 0] = b3f
    padinit = np.zeros((128, HALO), np.float32)
    ftinit = np.zeros((KB - NCH, NJ * CG), np.float32)
    return (_bf16(w1r), _bf16(w2r.reshape(K2, 6 * K2)),
            _bf16(w3r.reshape(K2, NJ * NCH)), b1v, b2v, b3v,
            _bf16(padinit), _bf16(ftinit))


def kernel(image, targets, w1, g1, b1, m1, v1, w2, g2, b2, m2, v2,
           w3, g3, b3, m3, v3):
    image = np.asarray(image, np.float32)
    targets = np.asarray(targets)
    w1r, w2r, w3r, b1v, b2v, b3v, padinit, ftinit = _prep_weights(
        np.asarray(w1, np.float32), np.asarray(g1, np.float32),
        np.asarray(b1, np.float32), np.asarray(m1, np.float32),
        np.asarray(v1, np.float32),
        np.asarray(w2, np.float32), np.asarray(g2, np.float32),
        np.asarray(b2, np.float32), np.asarray(m2, np.float32),
        np.asarray(v2, np.float32),
        np.asarray(w3, np.float32), np.asarray(g3, np.float32),
        np.asarray(b3, np.float32), np.asarray(m3, np.float32),
        np.asarray(v3, np.float32))

    lt = targets[:, :2].astype(np.int64)  # [512,2] (y,x)
    in_maps = []
    for c in range(NCORES):
        ci = lt[c * PC:(c + 1) * PC]
        crops = np.zeros((3, PC, PAD, PAD), ml_dtypes.bfloat16)
        for k, (y, x) in enumerate(ci):
            crops[:, k, 1:41, 1:41] = image[:, y:y + CROP, x:x + CROP]
        in_maps.append({
            "crops": crops,
            "w1r": w1r, "w2r": w2r, "w3r": w3r,
            "b1": b1v, "b2": b2v, "b3": b3v,
            "padinit": padinit, "ftinit": ftinit,
            "band_src": np.ascontiguousarray(
                image[:, c * BAND_H:(c + 1) * BAND_H, :]),
        })

    nc = _build()
    res_obj = run_bass_kernel_spmd(nc, in_maps, list(range(NCORES)),
                                   **RUN_KWARGS)
    globals()["LAST_RESULTS"] = res_obj
    res = res_obj.results

    out = np.empty_like(image)
    vals = np.empty((NGT, NCH), np.float32)
    for c in range(NCORES):
        out[:, c * BAND_H:(c + 1) * BAND_H, :] = res[c]["out_band"]
        vals[c * PC:(c + 1) * PC] = res[c]["vals_out"].T
    # host scatter of the dot values (unshard/assembly step)
    v = vals.reshape(NGT, 17, 3)
    coords = (lt[:, None, None, :] + DOT_LIST[None, :, None, :]
              + DIRS[None, None, :, :]).reshape(-1, 2)  # [512*17*9, 2]
    vflat = np.broadcast_to(v[:, :, None, :],
                            (NGT, 17, 9, 3)).reshape(-1, 3)
    out[:, coords[:, 0], coords[:, 1]] = vflat.T
    return out
